# revision 4
# baseline (speedup 1.0000x reference)
"""Trainium2 Bass kernel for nn_ClusteringLayer (vq_codebook).

Computes, for x (B,D) and clusters (K,D):
    sq   = ||x_i||^2 - 2 x.clusters^T + ||c_j||^2     (B,K)
    num  = 1/(1 + sqrt(sq))        (ALPHA=1 -> exponent -1)
    out  = num / sum(num)          (global scalar normalizer)

Sharding: data-parallel on batch across 8 NeuronCores; clusters
replicated; one 4-byte AllReduce for the normalizer.

Key structure (v2):
  - Host precomputes x2 = ||x_i||^2 (fp32, laid out [128,16] so it can
    be an ACT per-partition bias) and -c2/2 as a bf16 hi/lo pair that is
    folded into the PSUM accumulation via a K=2 ones-weight matmul.
    This removes all on-device square/row-reduce work.
  - num = 1/(1+sqrt(sq)) == sigmoid(-0.5*ln(sq)), so the per-element
    pipeline is two ACT table passes (Ln with scale=-2/bias=x2, then
    Sigmoid with scale=-0.5 + accum_out row sums). All 16 Ln passes run
    before the 16 Sigmoid passes so the ACT table set switches once.
  - The warmup AllReduce input is an ExternalInput DRAM scalar, so the
    gpsimd trigger fires at t~0 with no memset/DMA dependency. The ~50us
    ncfw wake then fully overlaps compute.
  - Final scale by 1/total is split across DVE/ACT/Pool, with one 1MB
    output DMA per 2 m-tiles.
"""

import numpy as np

B, D, K = 16384, 512, 1024
N_CORES = 8
BL = B // N_CORES        # 2048 rows per core
P = 128                  # partitions
MT = BL // P             # 16 m-tiles per core
KC = D // P              # 4 contraction chunks
NJ = 512                 # matmul moving free dim limit (one PSUM bank)
JH = K // NJ             # 2 j-halves

_CACHE = {}


def _build_bass():
    import concourse.bass as bass  # noqa: F401
    import concourse.mybir as mybir
    import concourse.tile as tile
    from concourse import bacc

    f32 = mybir.dt.float32
    bf16 = mybir.dt.bfloat16
    AF = mybir.ActivationFunctionType
    ALU = mybir.AluOpType

    nc = bacc.Bacc(
        "TRN2", target_bir_lowering=False, debug=False, num_devices=N_CORES
    )
    xT_d = nc.dram_tensor("xT", [D, BL], bf16, kind="ExternalInput").ap()
    cT_d = nc.dram_tensor("cT", [D, K], bf16, kind="ExternalInput").ap()
    c2f_d = nc.dram_tensor("c2f", [2, K], bf16, kind="ExternalInput").ap()
    x2_d = nc.dram_tensor("x2", [P, MT], f32, kind="ExternalInput").ap()
    wones_d = nc.dram_tensor("wones", [2, P], bf16, kind="ExternalInput").ap()
    onesc_d = nc.dram_tensor("onesc", [P, 1], f32, kind="ExternalInput").ap()
    onesr_d = nc.dram_tensor("onesr", [1, P], f32, kind="ExternalInput").ap()
    zin_d = nc.dram_tensor("zin", [1, 1], f32, kind="ExternalInput").ap()
    out_d = nc.dram_tensor("out", [BL, K], f32, kind="ExternalOutput").ap()

    with tile.TileContext(nc) as tc:
        with (
            tc.tile_pool(name="const", bufs=1) as cpool,
            tc.tile_pool(name="big", bufs=1) as bpool,
            tc.tile_pool(name="pmm", bufs=3, space="PSUM") as pmm,
            tc.tile_pool(name="prow", bufs=2, space="PSUM") as prow,
            tc.tile_pool(name="dram", bufs=1, space="DRAM") as dpool,
        ):
            # ---- warmup AllReduce, triggered immediately: input is an
            # ExternalInput already in DRAM, so the gpsimd trigger has no
            # deps. The ~50us cold ncfw wake overlaps all of compute.
            with tc.high_priority():
                cc_w_in = dpool.tile([1, 1], f32)
                nc.sync.dma_start(cc_w_in, zin_d)
                warm_out = dpool.tile([1, 1], f32, addr_space="Shared")
                nc.gpsimd.collective_compute(
                    "AllReduce",
                    ALU.add,
                    replica_groups=[list(range(N_CORES))],
                    ins=[cc_w_in.opt()],
                    outs=[warm_out.opt()],
                )

            # ---- constants / small inputs ----
            c2f = cpool.tile([2, K], bf16)          # [-c2/2 hi; -c2/2 lo]
            nc.sync.dma_start(c2f, c2f_d)
            wones = cpool.tile([2, P], bf16)        # ones lhsT for c2 fold
            nc.sync.dma_start(wones, wones_d)
            x2sb = cpool.tile([P, MT], f32)         # ACT bias source
            nc.sync.dma_start(x2sb, x2_d)
            onesc = cpool.tile([P, 1], f32)         # cross-partition sum lhsT
            nc.sync.dma_start(onesc, onesc_d)
            onesr = cpool.tile([1, P], f32)         # inv broadcast lhsT
            nc.sync.dma_start(onesr, onesr_d)

            # ---- stream inputs; first 4 m-tiles' x columns arrive first ----
            NA = 4 * P  # leading column group per chunk
            xTa, xTb, cTs = [], [], []
            for k in range(KC):
                ct = bpool.tile([P, K], bf16, name=f"cT{k}")
                nc.sync.dma_start(ct, cT_d[k * P : (k + 1) * P, :])
                cTs.append(ct)
                xa = bpool.tile([P, NA], bf16, name=f"xTa{k}")
                nc.sync.dma_start(xa, xT_d[k * P : (k + 1) * P, 0:NA])
                xTa.append(xa)
            for k in range(KC):
                xb = bpool.tile([P, BL - NA], bf16, name=f"xTb{k}")
                nc.sync.dma_start(xb, xT_d[k * P : (k + 1) * P, NA:BL])
                xTb.append(xb)

            Lbuf = bpool.tile([P, MT * K], f32)     # ln(sq), 64 KB/partition
            numbuf = bpool.tile([P, MT * K], f32)   # num, 64 KB/partition
            acc = cpool.tile([P, MT], f32)          # per-tile row sums

            # ---- main loop: 10 matmuls + 1 Ln pass per m-tile ----
            for i in range(MT):
                ps = pmm.tile([P, K], f32, tag="mm")
                for h in range(JH):
                    nc.tensor.matmul(
                        ps[:, h * NJ : (h + 1) * NJ],
                        lhsT=wones,
                        rhs=c2f[:, h * NJ : (h + 1) * NJ],
                        start=True,
                        stop=False,
                    )
                for k in range(KC):
                    if (i + 1) * P <= NA:
                        lhsT = xTa[k][:, i * P : (i + 1) * P]
                    else:
                        lhsT = xTb[k][:, i * P - NA : (i + 1) * P - NA]
                    for h in range(JH):
                        nc.tensor.matmul(
                            ps[:, h * NJ : (h + 1) * NJ],
                            lhsT=lhsT,
                            rhs=cTs[k][:, h * NJ : (h + 1) * NJ],
                            start=False,
                            stop=(k == KC - 1),
                        )
                # L = ln(-2*psum + x2)  (= ln(sq); sq >= ~650 always)
                nc.scalar.activation(
                    Lbuf[:, i * K : (i + 1) * K], ps, AF.Ln,
                    bias=x2sb[:, i : i + 1], scale=-2.0,
                )

            # ---- num = sigmoid(-0.5*L), with per-tile row sums ----
            for i in range(MT):
                nc.scalar.activation(
                    numbuf[:, i * K : (i + 1) * K],
                    Lbuf[:, i * K : (i + 1) * K],
                    AF.Sigmoid, bias=0.0, scale=-0.5,
                    accum_out=acc[:, i : i + 1],
                )

            # ---- local sum -> AllReduce -> invb [P,1] ----
            ps16 = prow.tile([1, MT], f32, tag="row")
            nc.tensor.matmul(ps16, lhsT=onesc, rhs=acc, start=True, stop=True)
            lsum = cpool.tile([1, 1], f32)
            nc.vector.reduce_sum(lsum, ps16, axis=mybir.AxisListType.X)
            cc_in = dpool.tile([1, 1], f32)
            cc_out = dpool.tile([1, 1], f32, addr_space="Shared")
            nc.sync.dma_start(cc_in, lsum)
            nc.gpsimd.collective_compute(
                "AllReduce",
                ALU.add,
                replica_groups=[list(range(N_CORES))],
                ins=[cc_in.opt()],
                outs=[cc_out.opt()],
            )
            tot = cpool.tile([1, 1], f32)
            nc.sync.dma_start(tot, cc_out)
            inv1 = cpool.tile([1, 1], f32)
            nc.vector.reciprocal(inv1, tot)
            psb = prow.tile([P, 1], f32, tag="row")
            nc.tensor.matmul(psb, lhsT=onesr, rhs=inv1, start=True, stop=True)
            invb = cpool.tile([P, 1], f32)
            nc.vector.tensor_copy(invb, psb)

            # ---- scale (DVE/ACT/Pool split) + 8x 1MB output DMAs ----
            GW = 2 * K  # columns per group = 2 m-tiles
            engines = ["v", "a", "p", "v", "a", "p", "v", "a"]
            for g in range(MT // 2):
                sl = numbuf[:, g * GW : (g + 1) * GW]
                eng = engines[g]
                if eng == "v":
                    nc.vector.tensor_scalar_mul(sl, sl, invb)
                    src = sl
                elif eng == "p":
                    nc.gpsimd.tensor_scalar_mul(sl, sl, invb)
                    src = sl
                else:
                    # ACT writes into Lbuf (dead after sigmoid) to avoid
                    # in-place ACT; Copy is in every table set.
                    dst = Lbuf[:, g * GW : (g + 1) * GW]
                    nc.scalar.activation(dst, sl, AF.Copy, scale=invb)
                    src = dst
                dstd = out_d[g * 2 * P : (g + 1) * 2 * P, :].rearrange(
                    "(f p) c -> p f c", p=P
                )
                nc.sync.dma_start(dstd, src.rearrange("p (f c) -> p f c", f=2))

    nc.finalize()
    return nc


def _get_bass():
    key = "nc"
    if key not in _CACHE:
        _CACHE[key] = _build_bass()
    return _CACHE[key]


def _host_prep(x: np.ndarray, clusters: np.ndarray):
    import ml_dtypes

    bf = ml_dtypes.bfloat16
    cT = np.ascontiguousarray(clusters.T).astype(bf)
    c2 = np.sum(clusters.astype(np.float64) ** 2, axis=1)  # (K,)
    mh = (-0.5 * c2).astype(np.float32)
    hi = mh.astype(bf)
    lo = (mh - hi.astype(np.float32)).astype(bf)
    c2f = np.ascontiguousarray(np.stack([hi, lo], axis=0))  # (2,K) bf16
    wones = np.ones((2, P), dtype=bf)
    onesc = np.ones((P, 1), dtype=np.float32)
    onesr = np.ones((1, P), dtype=np.float32)
    zin = np.zeros((1, 1), dtype=np.float32)

    x2_full = np.sum(x.astype(np.float64) ** 2, axis=1).astype(np.float32)
    in_maps = []
    for c in range(N_CORES):
        xs = x[c * BL : (c + 1) * BL]
        xT_c = np.ascontiguousarray(xs.T).astype(bf)
        # x2 laid out [P, MT]: x2[p, i] = ||x_row(i*128+p)||^2
        x2_c = np.ascontiguousarray(
            x2_full[c * BL : (c + 1) * BL].reshape(MT, P).T
        )
        in_maps.append({
            "xT": xT_c, "cT": cT, "c2f": c2f, "x2": x2_c,
            "wones": wones, "onesc": onesc, "onesr": onesr, "zin": zin,
        })
    return in_maps


def kernel(x: np.ndarray, clusters: np.ndarray) -> np.ndarray:
    from concourse.bass_utils import run_bass_kernel_spmd

    x = np.asarray(x, dtype=np.float32)
    clusters = np.asarray(clusters, dtype=np.float32)
    assert x.shape == (B, D) and clusters.shape == (K, D)

    in_maps = _host_prep(x, clusters)
    nc = _get_bass()
    res = run_bass_kernel_spmd(nc, in_maps, core_ids=list(range(N_CORES)))
    return np.concatenate([r["out"] for r in res.results], axis=0)


# revision 8
# speedup vs baseline: 1.5455x; 1.5455x over previous
"""Trainium2 Bass kernel for nn_ClusteringLayer (vq_codebook).

Computes, for x (B,D) and clusters (K,D):
    sq   = ||x_i||^2 - 2 x.clusters^T + ||c_j||^2     (B,K)
    num  = 1/(1 + sqrt(sq))        (ALPHA=1 -> exponent -1)
    out  = num / sum(num)          (global scalar normalizer)

Sharding: data-parallel on batch across 8 NeuronCores; clusters
replicated; one 4-byte AllReduce for the normalizer.

v3 design notes (informed by trace analysis):
  - The PE is frequency-throttled to ~1.2GHz on this part, so bf16
    512-col matmuls cost ~420ns. fp8e4m3 DoubleRow mode (2 contraction
    rows/PE-pass) cuts PE streaming time ~4x: per m-tile, 2 DoubleRow
    passes cover all 512 contraction rows per j-half.
  - -c2/2 is folded into PSUM by one extra DoubleRow pass whose lhsT
    weights are [2,2,1,0] and rhs rows are an fp8 3-term expansion of
    -c2/4-ish (values kept under fp8e4m3's +-240 range).
    x2 = ||x_i||^2 rides the ACT Sqrt pass as a per-partition fp32 bias
    (host supplies it [128,16]); dist = Sqrt(-2*psum + x2).
  - num = 1/(1+dist) via DVE: +1 (tensor_scalar), reciprocal_approx_fast,
    then a mult-by-1 pass whose accum_out yields per-tile row sums.
    Single ACT table set (Sqrt) -> no table reloads.
  - All engines pay a ~5.5us entry handshake, but Sync pays ~10.5us; the
    warmup-AllReduce trigger DMA and the first input DMAs are issued
    from gpsimd/vector/scalar so compute and (critically) the ncfw wake
    start ~6us earlier.
  - Final scale by 1/total runs on DVE+ACT only (gpsimd tensor ops are
    ~25x slower) writing bf16 into 8 dedicated staging tiles (avoids
    same-tile cross-engine write serialization and DMA WAR chains),
    each followed by its 0.5MB output DMA. Host upcasts bf16->fp32.
"""

import numpy as np

B, D, K = 16384, 512, 1024
N_CORES = 8
BL = B // N_CORES        # 2048 rows per core
P = 128                  # partitions
MT = BL // P             # 16 m-tiles per core
KC2 = 2                  # DoubleRow contraction chunks (256 rows each)
NJ = 512                 # matmul moving free dim limit (one PSUM bank)
JH = K // NJ             # 2 j-halves

_CACHE = {}


def _build_bass():
    import concourse.bass as bass  # noqa: F401
    import concourse.mybir as mybir
    import concourse.tile as tile
    from concourse import bacc

    f32 = mybir.dt.float32
    bf16 = mybir.dt.bfloat16
    fp8 = mybir.dt.float8e4
    AF = mybir.ActivationFunctionType
    ALU = mybir.AluOpType
    DR = mybir.MatmulPerfMode.DoubleRow

    nc = bacc.Bacc(
        "TRN2", target_bir_lowering=False, debug=False, num_devices=N_CORES
    )
    x8_d = nc.dram_tensor("x8", [KC2, P, 2, BL], fp8, kind="ExternalInput").ap()
    c8_d = nc.dram_tensor("c8", [KC2, P, 2, K], fp8, kind="ExternalInput").ap()
    c2f_d = nc.dram_tensor("c2f", [2, 2, K], fp8, kind="ExternalInput").ap()
    w2_d = nc.dram_tensor("w2", [2, 2, P], fp8, kind="ExternalInput").ap()
    x2_d = nc.dram_tensor("x2", [P, MT], f32, kind="ExternalInput").ap()
    onesc_d = nc.dram_tensor("onesc", [P, 1], f32, kind="ExternalInput").ap()
    onesr_d = nc.dram_tensor("onesr", [1, P], f32, kind="ExternalInput").ap()
    zin_d = nc.dram_tensor("zin", [1, 1], f32, kind="ExternalInput").ap()
    out_d = nc.dram_tensor("out", [BL, K], bf16, kind="ExternalOutput").ap()

    with tile.TileContext(nc) as tc:
        with (
            tc.tile_pool(name="const", bufs=1) as cpool,
            tc.tile_pool(name="big", bufs=1) as bpool,
            tc.tile_pool(name="dd", bufs=2) as dpp,
            tc.tile_pool(name="pmm", bufs=3, space="PSUM") as pmm,
            tc.tile_pool(name="prow", bufs=2, space="PSUM") as prow,
            tc.tile_pool(name="dram", bufs=1, space="DRAM") as dpool,
        ):
            # ---- warmup AllReduce ASAP: DMA issued from gpsimd (ready at
            # ~5.5us vs sync's ~10.5us); wakes ncfw whose barrier+first-op
            # pipe is ~50us and must overlap compute.
            with tc.high_priority():
                cc_w_in = dpool.tile([1, 1], f32)
                nc.gpsimd.dma_start(cc_w_in, zin_d)
                warm_out = dpool.tile([1, 1], f32, addr_space="Shared")
                nc.gpsimd.collective_compute(
                    "AllReduce",
                    ALU.add,
                    replica_groups=[list(range(N_CORES))],
                    ins=[cc_w_in.opt()],
                    outs=[warm_out.opt()],
                )

            # ---- consts + first input chunks from early-ready engines ----
            c2f = cpool.tile([2, 2, K], fp8)
            nc.scalar.dma_start(c2f, c2f_d)
            w2 = cpool.tile([2, 2, P], fp8)
            nc.scalar.dma_start(w2, w2_d)
            x2sb = cpool.tile([P, MT], f32)
            nc.gpsimd.dma_start(x2sb, x2_d)
            onesc = cpool.tile([P, 1], f32)
            nc.gpsimd.dma_start(onesc, onesc_d)
            onesr = cpool.tile([1, P], f32)
            nc.gpsimd.dma_start(onesr, onesr_d)

            NA = 4 * P  # leading x columns (m-tiles 0-3)
            c8s, x8a, x8b = [], [], []
            for q in range(KC2):
                ct = bpool.tile([P, 2, K], fp8, name=f"c8_{q}")
                nc.scalar.dma_start(ct, c8_d[q])
                c8s.append(ct)
                xa = bpool.tile([P, 2, NA], fp8, name=f"x8a{q}")
                nc.gpsimd.dma_start(xa, x8_d[q, :, :, 0:NA])
                x8a.append(xa)
            for q in range(KC2):
                xb = bpool.tile([P, 2, BL - NA], fp8, name=f"x8b{q}")
                nc.sync.dma_start(xb, x8_d[q, :, :, NA:BL])
                x8b.append(xb)

            numbuf = bpool.tile([P, MT * K], f32)   # 64 KB/partition
            acc = cpool.tile([P, MT], f32)          # per-tile row sums

            # ---- main loop ----
            for i in range(MT):
                ps = pmm.tile([P, K], f32, tag="mm")
                for h in range(JH):
                    psl = ps[:, h * NJ : (h + 1) * NJ]
                    nc.tensor.matmul(
                        psl, lhsT=w2, rhs=c2f[:, :, h * NJ : (h + 1) * NJ],
                        start=True, stop=False, perf_mode=DR,
                    )
                    for q in range(KC2):
                        if (i + 1) * P <= NA:
                            lhsT = x8a[q][:, :, i * P : (i + 1) * P]
                        else:
                            lhsT = x8b[q][:, :, i * P - NA : (i + 1) * P - NA]
                        nc.tensor.matmul(
                            psl, lhsT=lhsT,
                            rhs=c8s[q][:, :, h * NJ : (h + 1) * NJ],
                            start=False, stop=(q == KC2 - 1), perf_mode=DR,
                        )
                # dist = sqrt(-2*psum + x2); D ping-pongs over 2 bufs
                dtile = dpp.tile([P, K], f32, tag="d")
                nc.scalar.activation(
                    dtile, ps, AF.Sqrt, bias=x2sb[:, i : i + 1], scale=-2.0
                )
                nsl = numbuf[:, i * K : (i + 1) * K]
                nc.vector.tensor_scalar_add(dtile, dtile, 1.0)
                nc.vector.reciprocal_approx_fast(nsl, dtile)
                nc.vector.tensor_scalar(
                    nsl, nsl, 1.0, 0.0, ALU.mult, ALU.add,
                    accum_out=acc[:, i : i + 1],
                )

            # ---- local sum -> AllReduce -> invb [P,1] ----
            ps16 = prow.tile([1, MT], f32, tag="row")
            nc.tensor.matmul(ps16, lhsT=onesc, rhs=acc, start=True, stop=True)
            lsum = cpool.tile([1, 1], f32)
            nc.vector.reduce_sum(lsum, ps16, axis=mybir.AxisListType.X)
            cc_in = dpool.tile([1, 1], f32)
            cc_out = dpool.tile([1, 1], f32, addr_space="Shared")
            nc.sync.dma_start(cc_in, lsum)
            nc.gpsimd.collective_compute(
                "AllReduce",
                ALU.add,
                replica_groups=[list(range(N_CORES))],
                ins=[cc_in.opt()],
                outs=[cc_out.opt()],
            )
            tot = cpool.tile([1, 1], f32)
            nc.sync.dma_start(tot, cc_out)
            inv1 = cpool.tile([1, 1], f32)
            nc.vector.reciprocal(inv1, tot)
            psb = prow.tile([P, 1], f32, tag="row")
            nc.tensor.matmul(psb, lhsT=onesr, rhs=inv1, start=True, stop=True)
            invb = cpool.tile([P, 1], f32)
            nc.vector.tensor_copy(invb, psb)

            # ---- scale (DVE/ACT alternating) into dedicated bf16 staging
            # tiles, one 0.5MB DMA per group of 2 m-tiles ----
            GW = 2 * K
            for g in range(MT // 2):
                sl = numbuf[:, g * GW : (g + 1) * GW]
                st = bpool.tile([P, GW], bf16, name=f"stg{g}")
                if g % 2 == 0:
                    nc.vector.tensor_scalar_mul(st, sl, invb)
                else:
                    nc.scalar.activation(st, sl, AF.Copy, scale=invb)
                dstd = out_d[g * 2 * P : (g + 1) * 2 * P, :].rearrange(
                    "(f p) c -> p f c", p=P
                )
                nc.sync.dma_start(dstd, st.rearrange("p (f c) -> p f c", f=2))

    nc.finalize()
    return nc


def _get_bass():
    key = "nc"
    if key not in _CACHE:
        _CACHE[key] = _build_bass()
    return _CACHE[key]


def _f8(a):
    import concourse.mybir as mybir

    return a.astype(mybir.dt.np(mybir.dt.float8e4))


def _host_prep(x: np.ndarray, clusters: np.ndarray):
    f8np = _f8(np.zeros(1)).dtype

    # fp8 operands with DoubleRow plane layout [chunk][128][plane][cols]:
    # plane p of chunk q holds contraction rows q*256 + p*128 + (0..127).
    def dr_pack(mT):  # mT: (512, ncols) fp32 -> (2, 128, 2, ncols) fp8
        return np.ascontiguousarray(
            mT.reshape(KC2, 2, P, -1).transpose(0, 2, 1, 3)
        ).astype(f8np)

    cT = clusters.T.astype(np.float32)          # (512, 1024)
    c8 = dr_pack(cT)
    c2 = np.sum(clusters.astype(np.float64) ** 2, axis=1)
    t = (-0.5 * c2).astype(np.float32)          # in [-660, -200] roughly
    v0 = _f8(t * 0.5)
    r1 = t - 2.0 * v0.astype(np.float32)
    v1 = _f8(r1 * 0.5)
    r2 = r1 - 2.0 * v1.astype(np.float32)
    v2 = _f8(r2)
    zer = np.zeros_like(v2)
    # planes: [k=0,plane0]=v0 [k=1,plane0]=v1 [k=0,plane1]=v2 [k=1,plane1]=0
    c2f = np.ascontiguousarray(
        np.stack([np.stack([v0, v2]), np.stack([v1, zer])])
    )  # (2 part, 2 plane, K)
    w2 = np.zeros((2, 2, P), dtype=f8np)
    w2[0, 0, :] = 2.0   # v0 weight
    w2[1, 0, :] = 2.0   # v1 weight
    w2[0, 1, :] = 1.0   # v2 weight
    onesc = np.ones((P, 1), dtype=np.float32)
    onesr = np.ones((1, P), dtype=np.float32)
    zin = np.zeros((1, 1), dtype=np.float32)

    x2_full = np.sum(x.astype(np.float64) ** 2, axis=1).astype(np.float32)
    in_maps = []
    for c in range(N_CORES):
        xs = x[c * BL : (c + 1) * BL]
        x8_c = dr_pack(np.ascontiguousarray(xs.T))
        x2_c = np.ascontiguousarray(
            x2_full[c * BL : (c + 1) * BL].reshape(MT, P).T
        )
        in_maps.append({
            "x8": x8_c, "c8": c8, "c2f": c2f, "w2": w2, "x2": x2_c,
            "onesc": onesc, "onesr": onesr, "zin": zin,
        })
    return in_maps


def kernel(x: np.ndarray, clusters: np.ndarray) -> np.ndarray:
    from concourse.bass_utils import run_bass_kernel_spmd

    x = np.asarray(x, dtype=np.float32)
    clusters = np.asarray(clusters, dtype=np.float32)
    assert x.shape == (B, D) and clusters.shape == (K, D)

    in_maps = _host_prep(x, clusters)
    nc = _get_bass()
    res = run_bass_kernel_spmd(nc, in_maps, core_ids=list(range(N_CORES)))
    return np.concatenate(
        [r["out"].astype(np.float32) for r in res.results], axis=0
    )


# revision 14
# speedup vs baseline: 1.5674x; 1.0142x over previous
"""Trainium2 Bass kernel for nn_ClusteringLayer (vq_codebook).

Computes, for x (B,D) and clusters (K,D):
    sq   = ||x_i||^2 - 2 x.clusters^T + ||c_j||^2     (B,K)
    num  = 1/(1 + sqrt(sq))        (ALPHA=1 -> exponent -1)
    out  = num / sum(num)          (global scalar normalizer)

Sharding: data-parallel on batch across 8 NeuronCores; clusters
replicated; one 4-byte AllReduce for the normalizer.

v3 design notes (informed by trace analysis):
  - The PE is frequency-throttled to ~1.2GHz on this part, so bf16
    512-col matmuls cost ~420ns. fp8e4m3 DoubleRow mode (2 contraction
    rows/PE-pass) cuts PE streaming time ~4x: per m-tile, 2 DoubleRow
    passes cover all 512 contraction rows per j-half.
  - -c2/2 is folded into PSUM by one extra DoubleRow pass whose lhsT
    weights are [2,2,1,0] and rhs rows are an fp8 3-term expansion of
    -c2/4-ish (values kept under fp8e4m3's +-240 range).
    x2 = ||x_i||^2 rides the ACT Sqrt pass as a per-partition fp32 bias
    (host supplies it [128,16]); dist = Sqrt(-2*psum + x2).
  - num = 1/(1+dist) via DVE: +1 (tensor_scalar), reciprocal_approx_fast,
    then a mult-by-1 pass whose accum_out yields per-tile row sums.
    Single ACT table set (Sqrt) -> no table reloads.
  - All engines pay a ~5.5us entry handshake, but Sync pays ~10.5us; the
    warmup-AllReduce trigger DMA and the first input DMAs are issued
    from gpsimd/vector/scalar so compute and (critically) the ncfw wake
    start ~6us earlier.
  - Final scale by 1/total runs on DVE+ACT only (gpsimd tensor ops are
    ~25x slower) writing bf16 into 8 dedicated staging tiles (avoids
    same-tile cross-engine write serialization and DMA WAR chains),
    each followed by its 0.5MB output DMA. Host upcasts bf16->fp32.
"""

import numpy as np

B, D, K = 16384, 512, 1024
N_CORES = 8
BL = B // N_CORES        # 2048 rows per core
P = 128                  # partitions
MT = BL // P             # 16 m-tiles per core
KC2 = 2                  # DoubleRow contraction chunks (256 rows each)
NJ = 512                 # matmul moving free dim limit (one PSUM bank)
JH = K // NJ             # 2 j-halves

_CACHE = {}


def _build_bass():
    import concourse.bass as bass  # noqa: F401
    import concourse.mybir as mybir
    import concourse.tile as tile
    from concourse import bacc

    f32 = mybir.dt.float32
    bf16 = mybir.dt.bfloat16
    fp8 = mybir.dt.float8e4
    AF = mybir.ActivationFunctionType
    ALU = mybir.AluOpType
    DR = mybir.MatmulPerfMode.DoubleRow

    nc = bacc.Bacc(
        "TRN2", target_bir_lowering=False, debug=False, num_devices=N_CORES
    )
    x8_d = nc.dram_tensor("x8", [KC2, P, 2, BL], fp8, kind="ExternalInput").ap()
    c8_d = nc.dram_tensor("c8", [KC2, P, 2, K], fp8, kind="ExternalInput").ap()
    c2f_d = nc.dram_tensor("c2f", [2, 2, K], fp8, kind="ExternalInput").ap()
    w2_d = nc.dram_tensor("w2", [2, 2, P], fp8, kind="ExternalInput").ap()
    x2_d = nc.dram_tensor("x2", [P, MT], f32, kind="ExternalInput").ap()
    onesc_d = nc.dram_tensor("onesc", [P, 1], f32, kind="ExternalInput").ap()
    onesr_d = nc.dram_tensor("onesr", [1, P], f32, kind="ExternalInput").ap()
    out_d = nc.dram_tensor("out", [BL, K], bf16, kind="ExternalOutput").ap()

    with tile.TileContext(nc) as tc:
        with (
            tc.tile_pool(name="const", bufs=1) as cpool,
            tc.tile_pool(name="big", bufs=1) as bpool,
            tc.tile_pool(name="dd", bufs=2) as dpp,
            tc.tile_pool(name="pmm", bufs=3, space="PSUM") as pmm,
            tc.tile_pool(name="prow", bufs=2, space="PSUM") as prow,
            tc.tile_pool(name="dram", bufs=1, space="DRAM") as dpool,
        ):
            # ---- warmup AllReduce ASAP: its input is an internal DRAM tile
            # that is never written (value irrelevant, result unused), so the
            # gpsimd doorbell write has zero dependencies and fires right
            # after the ~6us engine entry handshake, uniformly on all cores.
            # The ncfw wake pipe (~67us doorbell->wake->barrier->gap) then
            # overlaps compute entirely.
            with tc.high_priority():
                cc_w_in = dpool.tile([1, 1], f32)
                warm_out = dpool.tile([1, 1], f32, addr_space="Shared")
                nc.gpsimd.collective_compute(
                    "AllReduce",
                    ALU.add,
                    replica_groups=[list(range(N_CORES))],
                    ins=[cc_w_in.opt()],
                    outs=[warm_out.opt()],
                )

            # ---- consts + first input chunks issued from the scalar engine
            # (ready ~5.8us and lands on a fast hardware DMA queue; gpsimd
            # descriptors go to the slow software queue, sync starts ~10.5us)
            c2f = cpool.tile([2, 2, K], fp8)
            nc.scalar.dma_start(c2f, c2f_d)
            w2 = cpool.tile([2, 2, P], fp8)
            nc.scalar.dma_start(w2, w2_d)

            NA = 4 * P  # leading x columns (m-tiles 0-3)
            c8s, x8a, x8b = [], [], []
            for q in range(KC2):
                xa = bpool.tile([P, 2, NA], fp8, name=f"x8a{q}")
                nc.scalar.dma_start(xa, x8_d[q, :, :, 0:NA])
                x8a.append(xa)
                ct = bpool.tile([P, 2, K], fp8, name=f"c8_{q}")
                nc.scalar.dma_start(ct, c8_d[q])
                c8s.append(ct)
            x2sb = cpool.tile([P, MT], f32)
            nc.scalar.dma_start(x2sb, x2_d)
            for q in range(KC2):
                xb = bpool.tile([P, 2, BL - NA], fp8, name=f"x8b{q}")
                nc.sync.dma_start(xb, x8_d[q, :, :, NA:BL])
                x8b.append(xb)
            onesc = cpool.tile([P, 1], f32)
            nc.sync.dma_start(onesc, onesc_d)
            onesr = cpool.tile([1, P], f32)
            nc.sync.dma_start(onesr, onesr_d)

            numbuf = bpool.tile([P, MT * K], f32)   # 64 KB/partition
            acc = cpool.tile([P, MT], f32)          # per-tile row sums

            # ---- main loop ----
            for i in range(MT):
                ps = pmm.tile([P, K], f32, tag="mm")
                for h in range(JH):
                    psl = ps[:, h * NJ : (h + 1) * NJ]
                    nc.tensor.matmul(
                        psl, lhsT=w2, rhs=c2f[:, :, h * NJ : (h + 1) * NJ],
                        start=True, stop=False, perf_mode=DR,
                    )
                    for q in range(KC2):
                        if (i + 1) * P <= NA:
                            lhsT = x8a[q][:, :, i * P : (i + 1) * P]
                        else:
                            lhsT = x8b[q][:, :, i * P - NA : (i + 1) * P - NA]
                        nc.tensor.matmul(
                            psl, lhsT=lhsT,
                            rhs=c8s[q][:, :, h * NJ : (h + 1) * NJ],
                            start=False, stop=(q == KC2 - 1), perf_mode=DR,
                        )
                # dist = sqrt(-2*psum + x2); D ping-pongs over 2 bufs
                dtile = dpp.tile([P, K], f32, tag="d")
                nc.scalar.activation(
                    dtile, ps, AF.Sqrt, bias=x2sb[:, i : i + 1], scale=-2.0
                )
                nsl = numbuf[:, i * K : (i + 1) * K]
                nc.vector.tensor_scalar_add(dtile, dtile, 1.0)
                nc.vector.reciprocal_approx_fast(nsl, dtile)
                nc.vector.tensor_scalar(
                    nsl, nsl, 1.0, 0.0, ALU.mult, ALU.add,
                    accum_out=acc[:, i : i + 1],
                )

            # ---- local sum -> AllReduce -> invb [P,1] ----
            ps16 = prow.tile([1, MT], f32, tag="row")
            nc.tensor.matmul(ps16, lhsT=onesc, rhs=acc, start=True, stop=True)
            lsum = cpool.tile([1, 1], f32)
            nc.vector.reduce_sum(lsum, ps16, axis=mybir.AxisListType.X)
            cc_in = dpool.tile([1, 1], f32)
            cc_out = dpool.tile([1, 1], f32, addr_space="Shared")
            nc.sync.dma_start(cc_in, lsum)
            nc.gpsimd.collective_compute(
                "AllReduce",
                ALU.add,
                replica_groups=[list(range(N_CORES))],
                ins=[cc_in.opt()],
                outs=[cc_out.opt()],
            )
            tot = cpool.tile([1, 1], f32)
            nc.sync.dma_start(tot, cc_out)
            inv1 = cpool.tile([1, 1], f32)
            nc.vector.reciprocal(inv1, tot)
            psb = prow.tile([P, 1], f32, tag="row")
            nc.tensor.matmul(psb, lhsT=onesr, rhs=inv1, start=True, stop=True)
            invb = cpool.tile([P, 1], f32)
            nc.vector.tensor_copy(invb, psb)

            # ---- scale (DVE/ACT alternating) into dedicated bf16 staging
            # tiles, one 0.5MB DMA per group of 2 m-tiles ----
            GW = 2 * K
            for g in range(MT // 2):
                sl = numbuf[:, g * GW : (g + 1) * GW]
                st = bpool.tile([P, GW], bf16, name=f"stg{g}")
                if g % 2 == 0:
                    nc.vector.tensor_scalar_mul(st, sl, invb)
                else:
                    nc.scalar.activation(st, sl, AF.Copy, scale=invb)
                dstd = out_d[g * 2 * P : (g + 1) * 2 * P, :].rearrange(
                    "(f p) c -> p f c", p=P
                )
                eng = nc.sync if g % 2 == 0 else nc.scalar
                eng.dma_start(dstd, st.rearrange("p (f c) -> p f c", f=2))

    nc.finalize()
    return nc


def _get_bass():
    key = "nc"
    if key not in _CACHE:
        _CACHE[key] = _build_bass()
    return _CACHE[key]


def _f8(a):
    import concourse.mybir as mybir

    return a.astype(mybir.dt.np(mybir.dt.float8e4))


def _host_prep(x: np.ndarray, clusters: np.ndarray):
    f8np = _f8(np.zeros(1)).dtype

    # fp8 operands with DoubleRow plane layout [chunk][128][plane][cols]:
    # plane p of chunk q holds contraction rows q*256 + p*128 + (0..127).
    def dr_pack(mT):  # mT: (512, ncols) fp32 -> (2, 128, 2, ncols) fp8
        return np.ascontiguousarray(
            mT.reshape(KC2, 2, P, -1).transpose(0, 2, 1, 3)
        ).astype(f8np)

    cT = clusters.T.astype(np.float32)          # (512, 1024)
    c8 = dr_pack(cT)
    c2 = np.sum(clusters.astype(np.float64) ** 2, axis=1)
    t = (-0.5 * c2).astype(np.float32)          # in [-660, -200] roughly
    v0 = _f8(t * 0.5)
    r1 = t - 2.0 * v0.astype(np.float32)
    v1 = _f8(r1 * 0.5)
    r2 = r1 - 2.0 * v1.astype(np.float32)
    v2 = _f8(r2)
    zer = np.zeros_like(v2)
    # planes: [k=0,plane0]=v0 [k=1,plane0]=v1 [k=0,plane1]=v2 [k=1,plane1]=0
    c2f = np.ascontiguousarray(
        np.stack([np.stack([v0, v2]), np.stack([v1, zer])])
    )  # (2 part, 2 plane, K)
    w2 = np.zeros((2, 2, P), dtype=f8np)
    w2[0, 0, :] = 2.0   # v0 weight
    w2[1, 0, :] = 2.0   # v1 weight
    w2[0, 1, :] = 1.0   # v2 weight
    onesc = np.ones((P, 1), dtype=np.float32)
    onesr = np.ones((1, P), dtype=np.float32)

    x2_full = np.sum(x.astype(np.float64) ** 2, axis=1).astype(np.float32)
    in_maps = []
    for c in range(N_CORES):
        xs = x[c * BL : (c + 1) * BL]
        x8_c = dr_pack(np.ascontiguousarray(xs.T))
        x2_c = np.ascontiguousarray(
            x2_full[c * BL : (c + 1) * BL].reshape(MT, P).T
        )
        in_maps.append({
            "x8": x8_c, "c8": c8, "c2f": c2f, "w2": w2, "x2": x2_c,
            "onesc": onesc, "onesr": onesr,
        })
    return in_maps


def kernel(x: np.ndarray, clusters: np.ndarray) -> np.ndarray:
    from concourse.bass_utils import run_bass_kernel_spmd

    x = np.asarray(x, dtype=np.float32)
    clusters = np.asarray(clusters, dtype=np.float32)
    assert x.shape == (B, D) and clusters.shape == (K, D)

    in_maps = _host_prep(x, clusters)
    nc = _get_bass()
    res = run_bass_kernel_spmd(nc, in_maps, core_ids=list(range(N_CORES)))
    return np.concatenate(
        [r["out"].astype(np.float32) for r in res.results], axis=0
    )
